# revision 1
# baseline (speedup 1.0000x reference)
"""Trainium2 Bass kernel for nn_Conv2d_selfAdapt (dense_cnn).

Math reduction (derived from the reference):
  The final einsum weight[(c*9+p), j] = KERN[p] is independent of output
  channel j, so all 512 output channels are identical:
      out[b, :, h, w] = S[b,h,w] - sum_p mask_p[b,h,w] * Sshift_p[b,h,w]
  where S = channel-sum of x, Sshift_p = zero-padded spatial shift of S,
  and mask = straight-through one-hot of argmax over the 8 gate channels
  (softmax is monotone, theta=1 -> argmax(LN(conv(x,w)) + gumbel)).

  The only heavy compute is the 3x3 conv (512 -> 8 ch).  It is computed as
  a 1x1 conv with M=73 outputs (9 taps x 8 ch, + a ones-row giving S),
  then the 9 per-tap partial maps are spatially shifted (via a zero-padded
  SBUF grid + shifted-window SBUF->SBUF DMAs) and summed with a K=72
  selection matmul.  Per-pixel LN/gumbel/argmax/select runs on the vector
  engine in a pixel-partition layout obtained with PE transposes.

Sharding: pure data parallel, 2 images per core across 8 cores.
"""

import os
import sys

import numpy as np

for _p in ("/opt/trn_rl_repo", "/root/.axon_site/_ro/trn_rl_repo"):
    if os.path.isdir(_p) and _p not in sys.path:
        sys.path.insert(0, _p)

import concourse.bass as bass
import concourse.bacc as bacc
import concourse.tile as tile
from concourse import mybir
from contextlib import ExitStack

B, C, H, W = 16, 512, 48, 48
N_CORES = 8
BPC = B // N_CORES          # images per core
HW = H * W                  # 2304
G = W + 2                   # padded grid side (50)
NBLK = HW // 128            # 18 pixel blocks per image
EPS_LN = 1e-6
BIG = 1000.0
FP = mybir.dt.float32
FR = mybir.dt.float32r
CHUNKS = [(0, 512), (512, 512), (1024, 512), (1536, 512), (2048, 256)]
RCHUNKS = [(0, 10), (10, 10), (20, 10), (30, 10), (40, 8)]   # (row0, nrows)

AL = mybir.AluOpType
AX = mybir.AxisListType


def build_nc(reps=1):
    nc = bacc.Bacc("TRN2", target_bir_lowering=False, debug=False,
                   num_devices=N_CORES)

    x_d = nc.dram_tensor("x", [BPC, C, HW], FP, kind="ExternalInput")
    g_d = nc.dram_tensor("g", [BPC, 8, HW], FP, kind="ExternalInput")
    w73_d = nc.dram_tensor("w73", [128, 4, 81], FP, kind="ExternalInput")
    sel_d = nc.dram_tensor("sel", [81, 17], FP, kind="ExternalInput")
    iota_d = nc.dram_tensor("iota", [128, 8], FP, kind="ExternalInput")
    iotab_d = nc.dram_tensor("iotab", [128, 8], FP, kind="ExternalInput")
    lnw_d = nc.dram_tensor("lnw", [128, 8], FP, kind="ExternalInput")
    lnb_d = nc.dram_tensor("lnb", [128, 8], FP, kind="ExternalInput")
    ones1_d = nc.dram_tensor("ones1", [1, 128], FP, kind="ExternalInput")
    ident_d = nc.dram_tensor("ident", [128, 128], FP, kind="ExternalInput")
    out_d = nc.dram_tensor("out", [BPC, C, HW], FP, kind="ExternalOutput")

    QHW = HW // 4            # 576 pixels = 12 rows per quarter-image
    QCHUNKS = [(0, 512), (512, 64)]

    with tile.TileContext(nc) as tc, ExitStack() as ctx:
        consts = ctx.enter_context(tc.tile_pool(name="consts", bufs=1))
        xpool = ctx.enter_context(tc.tile_pool(name="xp", bufs=2 * 4))
        work = ctx.enter_context(tc.tile_pool(name="work", bufs=2))
        vp = ctx.enter_context(tc.tile_pool(name="vp", bufs=2))
        psA = ctx.enter_context(tc.tile_pool(name="psA", bufs=2, space="PSUM"))
        psS = ctx.enter_context(tc.tile_pool(name="psS", bufs=4, space="PSUM"))

        w73 = consts.tile([128, 4, 81], FP, tag="w73")
        sel = consts.tile([81, 17], FP, tag="sel")
        iota = consts.tile([128, 8], FP, tag="iota")
        iotab = consts.tile([128, 8], FP, tag="iotab")
        lnw = consts.tile([128, 8], FP, tag="lnw")
        lnb = consts.tile([128, 8], FP, tag="lnb")
        ones1 = consts.tile([1, 128], FP, tag="ones1")
        ident = consts.tile([128, 128], FP, tag="ident")
        eps_t = consts.tile([128, 1], FP, tag="eps")
        nc.vector.memset(eps_t, EPS_LN)
        nc.sync.dma_start(out=w73, in_=w73_d[:])
        nc.sync.dma_start(out=sel, in_=sel_d[:])
        nc.sync.dma_start(out=iota, in_=iota_d[:])
        nc.sync.dma_start(out=iotab, in_=iotab_d[:])
        nc.sync.dma_start(out=lnw, in_=lnw_d[:])
        nc.sync.dma_start(out=lnb, in_=lnb_d[:])
        nc.sync.dma_start(out=ones1, in_=ones1_d[:])
        nc.sync.dma_start(out=ident, in_=ident_d[:])
        ones1r = consts.tile([1, 128], FR, tag="ones1r")
        nc.vector.tensor_copy(ones1r, ones1)
        warm_t = consts.tile([128, 1], FP, tag="warm")
        nc.scalar.activation(warm_t, eps_t, mybir.ActivationFunctionType.Sqrt,
                             bias=eps_t, scale=1.0)

        lnw_b = lnw.unsqueeze(1).broadcast_to([128, NBLK, 8])
        lnb_b = lnb.unsqueeze(1).broadcast_to([128, NBLK, 8])
        iota_b = iota.unsqueeze(1).broadcast_to([128, NBLK, 8])
        iotab_b = iotab.unsqueeze(1).broadcast_to([128, NBLK, 8])

        import contextlib
        loop_ctx = tc.For_i(0, reps, 1) if reps > 1 else contextlib.nullcontext()
        with loop_ctx:
            body(nc, tc, ctx, locals())

    nc.compile()
    return nc


def body(nc, tc, ctx, env):
    x_d = env["x_d"]; g_d = env["g_d"]; out_d = env["out_d"]
    w73 = env["w73"]; sel = env["sel"]; ident = env["ident"]
    eps_t = env["eps_t"]; ones1r = env["ones1r"]
    lnw_b = env["lnw_b"]; lnb_b = env["lnb_b"]
    iota_b = env["iota_b"]; iotab_b = env["iotab_b"]
    work = env["work"]; vp = env["vp"]; xpool = env["xpool"]
    psA = env["psA"]; psS = env["psS"]
    QHW = env["QHW"]; QCHUNKS = env["QCHUNKS"]
    if True:
        # ---- input loads (all upfront; Tile overlaps) -----------------------
        xt = []
        for b in range(BPC):
            row = []
            for kt in range(4):
                t = xpool.tile([128, HW], FP, tag="x")
                nc.sync.dma_start(out=t, in_=x_d[b, kt * 128:(kt + 1) * 128, :])
                row.append(t)
            xt.append(row)
        t25s = []
        for b in range(BPC):
            t25 = work.tile([25, HW], FP, tag="t25")
            t25s.append(t25)
            nc.sync.dma_start(out=t25[17:25, :], in_=g_d[b])

        grids = []
        for b in range(BPC):
            # ---- conv as 1x1 matmul, half-image PSUM tiles ------------------
            # m = t*9 + o for o<8: per-tap logits partials; m = t*9+8: ones
            # column -> S (duplicated per tap so each tap block of 9 rows is
            # contiguous for the shifted-window DMA below).
            grid = work.tile([81, G, G], FP, tag="grid")
            grids.append(grid)
            nc.scalar.memzero(grid[:, 0, :])
            nc.scalar.memzero(grid[:, G - 1, :])
            nc.scalar.memzero(grid[:, 1:G - 1, 0:1])
            nc.scalar.memzero(grid[:, 1:G - 1, G - 1:G])
            for q in range(4):
                P81 = psA.tile([81, QHW], FP, tag="A")
                for kt in range(4):
                    for (off, sz) in QCHUNKS:
                        nc.tensor.matmul(
                            P81[:, off:off + sz],
                            w73[:, kt, :],
                            xt[b][kt][:, q * QHW + off:q * QHW + off + sz],
                            start=(kt == 0),
                            stop=(kt == 3),
                        )
                # zero-bordered grid (padding emulates the conv SAME padding)
                nc.scalar.copy(
                    out=grid[:, 1 + q * 12:1 + q * 12 + 12, 1:1 + W],
                    in_=P81.rearrange("p (h w) -> p h w", w=W),
                )

        for b in range(BPC):
            grid = grids[b]
            # ---- shifted per-tap rows via SBUF->SBUF window DMAs ------------
            psh = work.tile([81, HW], FP, tag="psh")
            for t in range(9):
                ti, tj = divmod(t, 3)
                nc.sync.dma_start(
                    out=psh[t * 9:(t + 1) * 9, :],
                    in_=grid[t * 9:(t + 1) * 9, ti:ti + H, tj:tj + W],
                )

            t25 = t25s[b]
            # selection matmul: rows 0-7 = tap-summed logits, 8+t = Sp_t
            for (off, sz) in CHUNKS:
                Lc = psS.tile([17, 512], FP, tag="S")
                nc.tensor.matmul(Lc[:, 0:sz], sel, psh[:, off:off + sz],
                                 start=True, stop=True)
                nc.scalar.copy(out=t25[0:17, off:off + sz], in_=Lc[:, 0:sz])

        for b in range(BPC):
            t25 = t25s[b]
            # ---- transpose to pixel-partition layout ------------------------
            tt = work.tile([128, NBLK, 25], FP, tag="tt")
            for half in range(2):
                tp = psS.tile([128, 9, 32], FP, tag="S")
                for blk in range(9):
                    nc.tensor.transpose(
                        tp[:, blk, 0:25],
                        t25[:, (half * 9 + blk) * 128:(half * 9 + blk + 1) * 128],
                        ident[0:25, 0:25],
                    )
                nc.vector.tensor_copy(tt[:, half * 9:(half + 1) * 9, :],
                                      tp[:, :, 0:25])

            # ---- per-pixel LN + gumbel + argmax + neighbor select -----------
            Lap = tt[:, :, 0:8]
            mu = vp.tile([128, NBLK], FP, tag="mu")
            nc.vector.tensor_reduce(mu, Lap, axis=AX.X, op=AL.add)
            cen = vp.tile([128, NBLK, 8], FP, tag="cen")
            nc.vector.scalar_tensor_tensor(
                cen, in0=mu.unsqueeze(2).broadcast_to([128, NBLK, 8]),
                scalar=-1.0 / 8.0, in1=Lap, op0=AL.mult, op1=AL.add)
            sq = vp.tile([128, NBLK, 8], FP, tag="sq")
            nc.vector.tensor_tensor(sq, cen, cen, op=AL.mult)
            v8 = vp.tile([128, NBLK], FP, tag="v8")
            nc.vector.tensor_reduce(v8, sq, axis=AX.X, op=AL.add)
            sd = vp.tile([128, NBLK], FP, tag="sd")
            nc.scalar.activation(sd, v8, mybir.ActivationFunctionType.Sqrt,
                                 bias=eps_t, scale=1.0 / 8.0)
            rstd = vp.tile([128, NBLK], FP, tag="rstd")
            nc.vector.reciprocal(rstd, sd)
            rl = vp.tile([128, NBLK, 8], FP, tag="rl")
            nc.vector.tensor_tensor(
                rl, rstd.unsqueeze(2).broadcast_to([128, NBLK, 8]), lnw_b,
                op=AL.mult)
            z1 = vp.tile([128, NBLK, 8], FP, tag="z1")
            nc.vector.tensor_tensor(z1, cen, rl, op=AL.mult)
            lnbg = vp.tile([128, NBLK, 8], FP, tag="lnbg")
            nc.vector.tensor_tensor(lnbg, tt[:, :, 17:25], lnb_b, op=AL.add)
            z = vp.tile([128, NBLK, 8], FP, tag="z")
            nc.vector.tensor_tensor(z, z1, lnbg, op=AL.add)

            mx = vp.tile([128, NBLK], FP, tag="mx")
            nc.vector.tensor_reduce(mx, z, axis=AX.X, op=AL.max)
            eq = vp.tile([128, NBLK, 8], FP, tag="eq")
            nc.vector.tensor_tensor(
                eq, z, mx.unsqueeze(2).broadcast_to([128, NBLK, 8]),
                op=AL.is_equal)
            im = vp.tile([128, NBLK, 8], FP, tag="im")
            nc.vector.scalar_tensor_tensor(
                im, in0=eq, scalar=-BIG, in1=iotab_b, op0=AL.mult, op1=AL.add)
            am = vp.tile([128, NBLK], FP, tag="am")
            nc.vector.tensor_reduce(am, im, axis=AX.X, op=AL.min)
            hard = vp.tile([128, NBLK, 8], FP, tag="hard")
            nc.vector.tensor_tensor(
                hard, iota_b, am.unsqueeze(2).broadcast_to([128, NBLK, 8]),
                op=AL.is_equal)

            # pair one-hot lanes with the 8 non-center taps (skip center=12)
            prod = vp.tile([128, NBLK, 8], FP, tag="prod")
            nc.vector.tensor_tensor(prod[:, :, 0:4], hard[:, :, 0:4],
                                    tt[:, :, 8:12], op=AL.mult)
            nc.vector.tensor_tensor(prod[:, :, 4:8], hard[:, :, 4:8],
                                    tt[:, :, 13:17], op=AL.mult)
            selS = vp.tile([128, NBLK], FP, tag="selS")
            nc.vector.tensor_reduce(selS, prod, axis=AX.X, op=AL.add)
            outm = vp.tile([128, NBLK], FP, tag="outm")
            nc.vector.tensor_tensor(outm, tt[:, :, 12], selS, op=AL.subtract)

            # ---- collapse map to one row (fp32r), then K=1 broadcast --------
            btp = psS.tile([18, 128], FP, tag="S")
            nc.tensor.transpose(btp, outm, ident)
            s18r = vp.tile([18, 128], FR, tag="s18r")
            nc.vector.tensor_copy(s18r, btp)
            row1r = vp.tile([1, HW], FR, tag="row1r")
            nc.gpsimd.dma_start(out=row1r, in_=s18r)

            outsb = work.tile([128, HW], FP, tag="outsb")
            for ci, (off, sz) in enumerate(CHUNKS):
                bcc = psS.tile([128, 512], FP, tag="S")
                nc.tensor.matmul(bcc[:, 0:sz], ones1r, row1r[:, off:off + sz],
                                 start=True, stop=True)
                nc.scalar.copy(out=outsb[:, off:off + sz], in_=bcc[:, 0:sz])
                # store half-image slabs as soon as they are complete
                if ci == 2:
                    for ct in range(4):
                        nc.scalar.dma_start(
                            out=out_d[b, ct * 128:(ct + 1) * 128, 0:1536],
                            in_=outsb[:, 0:1536])
            for ct in range(4):
                nc.scalar.dma_start(
                    out=out_d[b, ct * 128:(ct + 1) * 128, 1536:HW],
                    in_=outsb[:, 1536:HW])


def host_inputs(x, mask_weight, ln_weight, ln_bias, gumbel_noise):
    """Build per-core input maps (numpy only)."""
    x = np.ascontiguousarray(x, dtype=np.float32).reshape(B, C, HW)
    g = np.ascontiguousarray(gumbel_noise, dtype=np.float32).reshape(B, 8, HW)

    mw = np.asarray(mask_weight, dtype=np.float32).reshape(8, C, 9)
    a = mw.transpose(1, 2, 0)                         # [c, tap, o]
    w73 = np.ones((C, 9, 9), dtype=np.float32)        # [c, tap, o|ones]
    w73[:, :, :8] = a
    w73 = w73.reshape(4, 128, 81).transpose(1, 0, 2)  # [c_mod, kt, m]
    w73 = np.ascontiguousarray(w73)

    sel = np.zeros((81, 17), dtype=np.float32)
    for t in range(9):
        for o in range(8):
            sel[t * 9 + o, o] = 1.0
        sel[t * 9 + 8, 8 + t] = 1.0
    iota = np.broadcast_to(np.arange(8, dtype=np.float32), (128, 8)).copy()
    iotab = iota + BIG
    lnw = np.broadcast_to(
        np.asarray(ln_weight, np.float32).reshape(8), (128, 8)).copy()
    lnb = np.broadcast_to(
        np.asarray(ln_bias, np.float32).reshape(8), (128, 8)).copy()
    ident = np.eye(128, dtype=np.float32)

    shared = dict(w73=w73, sel=sel, iota=iota, iotab=iotab, lnw=lnw,
                  lnb=lnb, ones1=np.ones((1, 128), dtype=np.float32),
                  ident=ident)
    in_maps = []
    for c in range(N_CORES):
        m = dict(shared)
        m["x"] = np.ascontiguousarray(x[c * BPC:(c + 1) * BPC])
        m["g"] = np.ascontiguousarray(g[c * BPC:(c + 1) * BPC])
        in_maps.append(m)
    return in_maps


_NC = None


def kernel(x, mask_weight, ln_weight, ln_bias, gumbel_noise, init_flag=None,
           **_ignored):
    global _NC
    from concourse.bass_utils import run_bass_kernel_spmd

    if _NC is None:
        _NC = build_nc()
    in_maps = host_inputs(x, mask_weight, ln_weight, ln_bias, gumbel_noise)
    res = run_bass_kernel_spmd(_NC, in_maps, list(range(N_CORES))).results

    out = np.empty((B, C, H, W), dtype=np.float32)
    for c in range(N_CORES):
        out[c * BPC:(c + 1) * BPC] = res[c]["out"].reshape(BPC, C, H, W)
    return out



# revision 9
# speedup vs baseline: 1.1990x; 1.1990x over previous
"""Trainium2 Bass kernel for nn_Conv2d_selfAdapt (dense_cnn).

Math reduction (derived from the reference):
  The final einsum weight[(c*9+p), j] = KERN[p] is independent of output
  channel j, so all 512 output channels are identical:
      out[b, :, h, w] = S[b,h,w] - sum_p mask_p[b,h,w] * Sshift_p[b,h,w]
  where S = channel-sum of x, Sshift_p = zero-padded spatial shift of S,
  and mask = straight-through one-hot of argmax over the 8 gate channels
  (softmax is monotone, theta=1 -> argmax(LN(conv(x,w)) + gumbel)).

  The only heavy compute is the 3x3 conv (512 -> 8 ch).  It is computed as
  a 1x1 conv with M=81 outputs (9 taps x 8 ch + a ones-row giving S, taps
  duplicated) in fp32r (full PE rate), then the 9 per-tap partial maps are
  spatially shifted (zero-padded SBUF grid + shifted-window SBUF DMAs) and
  summed with a K=81 selection matmul (fp32r).  Per-pixel LN/gumbel/argmax/
  select runs on the vector engine in a pixel-partition layout obtained
  with 4 wide PE transposes per image (exact fp32 to protect the argmax).

  The device emits only the single [H*W] map per image; the host
  broadcasts it to the 512 identical output channels.

Sharding: pure data parallel, 2 images per core across 8 cores.
"""

import os
import sys

import numpy as np

for _p in ("/opt/trn_rl_repo", "/root/.axon_site/_ro/trn_rl_repo"):
    if os.path.isdir(_p) and _p not in sys.path:
        sys.path.insert(0, _p)

import concourse.bass as bass
import concourse.bacc as bacc
import concourse.tile as tile
from concourse import mybir
from contextlib import ExitStack

B, C, H, W = 16, 512, 48, 48
N_CORES = 8
BPC = B // N_CORES          # images per core
HW = H * W                  # 2304
G = W + 2                   # padded grid side (50)
EPS_LN = 1e-6
BIG = 1000.0
FP = mybir.dt.float32
FR = mybir.dt.float32r
# conv chunks: 48-col row-aligned so the PSUM->grid copy is rectangular
CCHUNKS = [(0, 10), (10, 10), (20, 10), (30, 10), (40, 8)]  # (row0, nrows)
# selection/pack chunks: 128-px block aligned (4x512 + 256)
SCHUNKS = [(0, 512), (512, 512), (1024, 512), (1536, 512), (2048, 256)]
NBLK = HW // 128            # 18 valid pixel blocks per image (20 padded)

AL = mybir.AluOpType
AX = mybir.AxisListType


def build_nc():
    nc = bacc.Bacc("TRN2", target_bir_lowering=False, debug=False,
                   num_devices=N_CORES)

    x_d = nc.dram_tensor("x", [BPC, C, HW], FR, kind="ExternalInput")
    g_d = nc.dram_tensor("g", [BPC, 8, HW], FP, kind="ExternalInput")
    w73_d = nc.dram_tensor("w73", [128, 4, 81], FR, kind="ExternalInput")
    sel_d = nc.dram_tensor("sel", [81, 17], FR, kind="ExternalInput")
    iota_d = nc.dram_tensor("iota", [128, 8], FP, kind="ExternalInput")
    iotab_d = nc.dram_tensor("iotab", [128, 8], FP, kind="ExternalInput")
    lnw_d = nc.dram_tensor("lnw", [128, 8], FP, kind="ExternalInput")
    ident_d = nc.dram_tensor("ident", [128, 128], FP, kind="ExternalInput")
    out_d = nc.dram_tensor("out", [BPC, HW], FP, kind="ExternalOutput")

    with tile.TileContext(nc) as tc, ExitStack() as ctx:
        consts = ctx.enter_context(tc.tile_pool(name="consts", bufs=1))
        xpool = ctx.enter_context(tc.tile_pool(name="xp", bufs=2 * 4))
        work = ctx.enter_context(tc.tile_pool(name="work", bufs=2))
        vp = ctx.enter_context(tc.tile_pool(name="vp", bufs=2))
        psA = ctx.enter_context(tc.tile_pool(name="psA", bufs=2, space="PSUM"))
        psS = ctx.enter_context(tc.tile_pool(name="psS", bufs=2, space="PSUM"))
        psT = ctx.enter_context(tc.tile_pool(name="psT", bufs=2, space="PSUM"))
        psO = ctx.enter_context(tc.tile_pool(name="psO", bufs=2, space="PSUM"))

        w73 = consts.tile([128, 4, 81], FR, tag="w73")
        sel = consts.tile([81, 17], FR, tag="sel")
        iota = consts.tile([128, 8], FP, tag="iota")
        iotab = consts.tile([128, 8], FP, tag="iotab")
        lnw = consts.tile([128, 8], FP, tag="lnw")
        ident = consts.tile([128, 128], FP, tag="ident")
        eps_t = consts.tile([128, 1], FP, tag="eps")
        nc.vector.memset(eps_t, EPS_LN)
        nc.sync.dma_start(out=w73, in_=w73_d[:])
        nc.sync.dma_start(out=sel, in_=sel_d[:])
        nc.sync.dma_start(out=iota, in_=iota_d[:])
        nc.sync.dma_start(out=iotab, in_=iotab_d[:])
        nc.sync.dma_start(out=lnw, in_=lnw_d[:])
        nc.sync.dma_start(out=ident, in_=ident_d[:])
        # warm the Sqrt activation table before the timed pipeline needs it
        warm_t = consts.tile([128, 1], FP, tag="warm")
        nc.scalar.activation(warm_t, eps_t, mybir.ActivationFunctionType.Sqrt,
                             bias=eps_t, scale=1.0)

        lnw_b = lnw.unsqueeze(1).broadcast_to([128, 20, 8])
        iota_b = iota.unsqueeze(1).broadcast_to([128, 20, 8])
        iotab_b = iotab.unsqueeze(1).broadcast_to([128, 20, 8])

        # ---- input loads (quarter-split for DMA/compute pipelining) --------
        xt = []
        for b in range(BPC):
            row = []
            for kt in range(4):
                t = xpool.tile([128, HW], FR, tag="x")
                row.append(t)
            xt.append(row)
        for b in range(BPC):
            for q in range(4):
                for kt in range(4):
                    nc.sync.dma_start(
                        out=xt[b][kt][:, q * 576:(q + 1) * 576],
                        in_=x_d[b, kt * 128:(kt + 1) * 128,
                                q * 576:(q + 1) * 576])

        # packed layout: 5 row-groups of 25 rows (17 sel outputs + 8 gumbel);
        # row-group rg covers pixels [512*rg, 512*rg+512)
        packs = []
        for b in range(BPC):
            packed = work.tile([125, 512], FP, tag="packed")
            packs.append(packed)
            # pixel blocks 18,19 don't exist; zero them so the vector
            # pipeline sees benign values in the padded lanes (rows 96:100
            # are re-covered by the rg=3 gumbel DMA afterwards)
            nc.scalar.memzero(packed[96:125, 256:512])
            for rg, (off, ncol) in enumerate(SCHUNKS):
                nc.sync.dma_start(
                    out=packed[25 * rg + 17:25 * rg + 25, 0:ncol],
                    in_=g_d[b, :, off:off + ncol])

        for b in range(BPC):
            # ---- conv as 1x1 matmul in fp32r, row-aligned chunks ----------
            grid = work.tile([81, G, G], FR, tag="grid")
            nc.scalar.memzero(grid[:, 0, :])
            nc.scalar.memzero(grid[:, G - 1, :])
            nc.scalar.memzero(grid[:, 1:G - 1, 0:1])
            nc.scalar.memzero(grid[:, 1:G - 1, G - 1:G])
            for (r0, nr) in CCHUNKS:
                ncol = nr * 48
                P = psA.tile([81, 480], FP, tag="A")
                for kt in range(4):
                    nc.tensor.matmul(
                        P[:, 0:ncol],
                        w73[:, kt, :],
                        xt[b][kt][:, r0 * 48:r0 * 48 + ncol],
                        start=(kt == 0),
                        stop=(kt == 3),
                    )
                nc.scalar.copy(
                    out=grid[:, 1 + r0:1 + r0 + nr, 1:1 + W],
                    in_=P[:, 0:ncol].rearrange("p (h w) -> p h w", w=W),
                )

            # ---- shifted per-tap rows via SBUF->SBUF window DMAs ----------
            psh = work.tile([81, HW], FR, tag="psh")
            for t in range(9):
                ti, tj = divmod(t, 3)
                nc.sync.dma_start(
                    out=psh[t * 9:(t + 1) * 9, :],
                    in_=grid[t * 9:(t + 1) * 9, ti:ti + H, tj:tj + W],
                )

            # ---- selection matmul (fp32r): rows 0-7 logits, 8+t = Sp_t ----
            packed = packs[b]
            for rg, (off, ncol) in enumerate(SCHUNKS):
                Lc = psS.tile([17, 512], FP, tag="S")
                nc.tensor.matmul(Lc[:, 0:ncol], sel,
                                 psh[:, off:off + ncol],
                                 start=True, stop=True)
                # compute engines can only start at partition 0/32/64/96,
                # so hop PSUM -> SBUF staging -> (DMA) packed row 25*rg
                stage = work.tile([17, 512], FP, tag="stage")
                nc.scalar.copy(out=stage[:, 0:ncol], in_=Lc[:, 0:ncol])
                nc.gpsimd.dma_start(
                    out=packed[25 * rg:25 * rg + 17, 0:ncol],
                    in_=stage[:, 0:ncol])

            # ---- transpose to pixel-partition layout (exact fp32) ---------
            # tt free layout [rg, cg, 25] so block index k = 4*rg+cg equals
            # the pixel-block index -> output rows come out in order
            tt = work.tile([128, 5, 4, 25], FP, tag="tt")
            for cg in range(4):
                tp = psT.tile([128, 125], FP, tag="T")
                nc.tensor.transpose(tp,
                                    packed[:, cg * 128:(cg + 1) * 128],
                                    ident[0:125, 0:125])
                nc.vector.tensor_copy(
                    tt[:, :, cg, :],
                    tp.rearrange("p (rg j) -> p rg j", j=25))
            ttv = tt.rearrange("p rg cg j -> p (rg cg) j")

            # ---- per-pixel LN + gumbel + argmax + neighbor select ---------
            Lap = ttv[:, :, 0:8]
            mu = vp.tile([128, 20], FP, tag="mu")
            nc.vector.tensor_reduce(mu, Lap, axis=AX.X, op=AL.add)
            cen = vp.tile([128, 20, 8], FP, tag="cen")
            nc.vector.scalar_tensor_tensor(
                cen, in0=mu.unsqueeze(2).broadcast_to([128, 20, 8]),
                scalar=-1.0 / 8.0, in1=Lap, op0=AL.mult, op1=AL.add)
            sq = vp.tile([128, 20, 8], FP, tag="sq")
            nc.vector.tensor_tensor(sq, cen, cen, op=AL.mult)
            v8 = vp.tile([128, 20], FP, tag="v8")
            nc.vector.tensor_reduce(v8, sq, axis=AX.X, op=AL.add)
            sd = vp.tile([128, 20], FP, tag="sd")
            nc.scalar.activation(sd, v8, mybir.ActivationFunctionType.Sqrt,
                                 bias=eps_t, scale=1.0 / 8.0)
            rstd = vp.tile([128, 20], FP, tag="rstd")
            nc.vector.reciprocal(rstd, sd)
            rl = vp.tile([128, 20, 8], FP, tag="rl")
            nc.vector.tensor_tensor(
                rl, rstd.unsqueeze(2).broadcast_to([128, 20, 8]), lnw_b,
                op=AL.mult)
            z = vp.tile([128, 20, 8], FP, tag="z")
            nc.vector.tensor_tensor(z, cen, rl, op=AL.mult)
            # add gumbel (+ln_bias, folded on host)
            z2 = vp.tile([128, 20, 8], FP, tag="z2")
            nc.vector.tensor_tensor(z2, z, ttv[:, :, 17:25], op=AL.add)

            mx = vp.tile([128, 20], FP, tag="mx")
            nc.vector.tensor_reduce(mx, z2, axis=AX.X, op=AL.max)
            eq = vp.tile([128, 20, 8], FP, tag="eq")
            nc.vector.tensor_tensor(
                eq, z2, mx.unsqueeze(2).broadcast_to([128, 20, 8]),
                op=AL.is_equal)
            im = vp.tile([128, 20, 8], FP, tag="im")
            nc.vector.scalar_tensor_tensor(
                im, in0=eq, scalar=-BIG, in1=iotab_b, op0=AL.mult, op1=AL.add)
            am = vp.tile([128, 20], FP, tag="am")
            nc.vector.tensor_reduce(am, im, axis=AX.X, op=AL.min)
            hard = vp.tile([128, 20, 8], FP, tag="hard")
            nc.vector.tensor_tensor(
                hard, iota_b, am.unsqueeze(2).broadcast_to([128, 20, 8]),
                op=AL.is_equal)

            # pair one-hot lanes with the 8 non-center taps (skip center=12)
            prod = vp.tile([128, 20, 8], FP, tag="prod")
            nc.vector.tensor_tensor(prod[:, :, 0:4], hard[:, :, 0:4],
                                    ttv[:, :, 8:12], op=AL.mult)
            nc.vector.tensor_tensor(prod[:, :, 4:8], hard[:, :, 4:8],
                                    ttv[:, :, 13:17], op=AL.mult)
            selS = vp.tile([128, 20], FP, tag="selS")
            nc.vector.tensor_reduce(selS, prod, axis=AX.X, op=AL.add)
            outm = vp.tile([128, 20], FP, tag="outm")
            nc.vector.tensor_tensor(outm, ttv[:, :, 12], selS,
                                    op=AL.subtract)

            # ---- transpose map back to pixel order and store --------------
            po = psO.tile([20, 128], FP, tag="O")
            nc.tensor.transpose(po, outm, ident)
            outsb = vp.tile([20, 128], FP, tag="outsb")
            nc.scalar.copy(out=outsb, in_=po)
            nc.sync.dma_start(
                out=out_d[b, :].rearrange("(j f) -> j f", f=128),
                in_=outsb[0:18, :])

    nc.compile()
    return nc


def _to_fp32r(a):
    """Round fp32 -> fp32r (11-bit mantissa, low 12 bits zero, RNE).

    The PE's fp32r datapath reads only the top 20 bits; pre-rounding on
    the host lets the kernel DMA the data straight into float32r tiles.
    """
    u = np.ascontiguousarray(a, dtype=np.float32).view(np.uint32)
    r = (u + 0x7FF + ((u >> 12) & 1)) & np.uint32(0xFFFFF000)
    return r.view(np.float32)


def host_inputs(x, mask_weight, ln_weight, ln_bias, gumbel_noise):
    """Build per-core input maps (numpy only)."""
    x = _to_fp32r(np.asarray(x, dtype=np.float32)).reshape(B, C, HW)
    g = np.asarray(gumbel_noise, dtype=np.float32).reshape(B, 8, HW)
    # fold the LN bias into the (precomputed) gumbel noise
    g = np.ascontiguousarray(
        g + np.asarray(ln_bias, np.float32).reshape(1, 8, 1))

    mw = np.asarray(mask_weight, dtype=np.float32).reshape(8, C, 9)
    a = mw.transpose(1, 2, 0)                         # [c, tap, o]
    w73 = np.ones((C, 9, 9), dtype=np.float32)        # [c, tap, o|ones]
    w73[:, :, :8] = a
    w73 = w73.reshape(4, 128, 81).transpose(1, 0, 2)  # [c_mod, kt, m]
    w73 = _to_fp32r(w73)

    sel = np.zeros((81, 17), dtype=np.float32)
    for t in range(9):
        for o in range(8):
            sel[t * 9 + o, o] = 1.0
        sel[t * 9 + 8, 8 + t] = 1.0
    iota = np.broadcast_to(np.arange(8, dtype=np.float32), (128, 8)).copy()
    iotab = iota + BIG
    lnw = np.broadcast_to(
        np.asarray(ln_weight, np.float32).reshape(8), (128, 8)).copy()
    ident = np.eye(128, dtype=np.float32)

    shared = dict(w73=w73, sel=sel, iota=iota, iotab=iotab, lnw=lnw,
                  ident=ident)
    in_maps = []
    for c in range(N_CORES):
        m = dict(shared)
        m["x"] = np.ascontiguousarray(x[c * BPC:(c + 1) * BPC])
        m["g"] = np.ascontiguousarray(g[c * BPC:(c + 1) * BPC])
        in_maps.append(m)
    return in_maps


_NC = None


def kernel(x, mask_weight, ln_weight, ln_bias, gumbel_noise, init_flag=None,
           **_ignored):
    global _NC
    from concourse.bass_utils import run_bass_kernel_spmd

    if _NC is None:
        _NC = build_nc()
    in_maps = host_inputs(x, mask_weight, ln_weight, ln_bias, gumbel_noise)
    res = run_bass_kernel_spmd(_NC, in_maps, list(range(N_CORES))).results

    # all 512 output channels are identical: broadcast the per-image map
    out = np.empty((B, C, H, W), dtype=np.float32)
    for c in range(N_CORES):
        maps = res[c]["out"].reshape(BPC, H, W)
        out[c * BPC:(c + 1) * BPC] = maps[:, None, :, :]
    return out


# revision 13
# speedup vs baseline: 1.7354x; 1.4474x over previous
"""Trainium2 Bass kernel for nn_Conv2d_selfAdapt (dense_cnn).

Math reduction (derived from the reference):
  The final einsum weight[(c*9+p), j] = KERN[p] is independent of output
  channel j, so all 512 output channels are identical:
      out[b, :, h, w] = S[b,h,w] - sum_p mask_p[b,h,w] * Sshift_p[b,h,w]
  where S = channel-sum of x, Sshift_p = zero-padded spatial shift of S,
  and mask = straight-through one-hot of argmax over the 8 gate channels
  (softmax is monotone, theta=1 -> argmax(LN(conv(x,w)) + gumbel)).

  The only heavy compute is the 3x3 conv (512 -> 8 ch).  It is computed as
  a 1x1 conv with M=81 outputs (9 taps x 8 ch + a ones-row giving S, taps
  duplicated) in fp32r (full PE rate), then the 9 per-tap partial maps are
  spatially shifted (zero-padded SBUF grid + shifted-window SBUF DMAs) and
  summed with a K=81 selection matmul (fp32r).  Per-pixel LN/gumbel/argmax/
  select runs on the vector engine in a pixel-partition layout obtained
  with 4 wide PE transposes per image (exact fp32 to protect the argmax).

  The device emits only the single [H*W] map per image; the host
  broadcasts it to the 512 identical output channels.

Sharding: pure data parallel, 2 images per core across 8 cores.
"""

import os
import sys

import numpy as np

for _p in ("/opt/trn_rl_repo", "/root/.axon_site/_ro/trn_rl_repo"):
    if os.path.isdir(_p) and _p not in sys.path:
        sys.path.insert(0, _p)

import concourse.bass as bass
import concourse.bacc as bacc
import concourse.tile as tile
from concourse import mybir
from contextlib import ExitStack

B, C, H, W = 16, 512, 48, 48
N_CORES = 8
BPC = B // N_CORES          # images per core
HW = H * W                  # 2304
G = W + 2                   # padded grid side (50)
EPS_LN = 1e-6
BIG = 1000.0
FP = mybir.dt.float32
FR = mybir.dt.float32r
# conv/sel chunks: 48-col row-aligned so PSUM copies stay rectangular
CCHUNKS = [(0, 10), (10, 10), (20, 10), (30, 10), (40, 8)]  # (row0, nrows)
# pack chunks: 128-px block aligned (4x512 + 256) for the transposes
PCHUNKS = [(0, 512), (512, 512), (1024, 512), (1536, 512), (2048, 256)]
NBLK = HW // 128            # 18 valid pixel blocks per image (20 padded)
# flat padded conv-partial layout: row pitch 49 (48 data + 1 zero spacer),
# data cell (r, w) at 50 + 49*r + w; head [0:50] and tail [2402:2452] zero.
GP = 49
GBASE = 50
GTOT = GBASE + GP * 48 + GP + 1  # 2452
PSHW = GP * 48               # 2352 (shifted window length per tap)

AL = mybir.AluOpType
AX = mybir.AxisListType


def build_nc():
    nc = bacc.Bacc("TRN2", target_bir_lowering=False, debug=False,
                   num_devices=N_CORES)

    x_d = nc.dram_tensor("x", [BPC, C, HW], FR, kind="ExternalInput")
    g_d = nc.dram_tensor("g", [BPC, 8, HW], FP, kind="ExternalInput")
    w73_d = nc.dram_tensor("w73", [128, 4, 81], FR, kind="ExternalInput")
    sel_d = nc.dram_tensor("sel", [81, 17], FR, kind="ExternalInput")
    iota_d = nc.dram_tensor("iota", [128, 8], FP, kind="ExternalInput")
    iotab_d = nc.dram_tensor("iotab", [128, 8], FP, kind="ExternalInput")
    lnw_d = nc.dram_tensor("lnw", [128, 8], FP, kind="ExternalInput")
    ident_d = nc.dram_tensor("ident", [128, 128], FP, kind="ExternalInput")
    out_d = nc.dram_tensor("out", [BPC, HW], FP, kind="ExternalOutput")

    with tile.TileContext(nc) as tc, ExitStack() as ctx:
        consts = ctx.enter_context(tc.tile_pool(name="consts", bufs=1))
        xpool = ctx.enter_context(tc.tile_pool(name="xp", bufs=2 * 4))
        work = ctx.enter_context(tc.tile_pool(name="work", bufs=2))
        vp = ctx.enter_context(tc.tile_pool(name="vp", bufs=2))
        psA = ctx.enter_context(tc.tile_pool(name="psA", bufs=2, space="PSUM"))
        psS = ctx.enter_context(tc.tile_pool(name="psS", bufs=2, space="PSUM"))
        psT = ctx.enter_context(tc.tile_pool(name="psT", bufs=2, space="PSUM"))
        psO = ctx.enter_context(tc.tile_pool(name="psO", bufs=2, space="PSUM"))

        w73 = consts.tile([128, 4, 81], FR, tag="w73")
        sel = consts.tile([81, 17], FR, tag="sel")
        iota = consts.tile([128, 8], FP, tag="iota")
        iotab = consts.tile([128, 8], FP, tag="iotab")
        lnw = consts.tile([128, 8], FP, tag="lnw")
        ident = consts.tile([128, 128], FP, tag="ident")
        eps_t = consts.tile([128, 1], FP, tag="eps")
        nc.vector.memset(eps_t, EPS_LN)
        nc.sync.dma_start(out=w73, in_=w73_d[:])
        nc.sync.dma_start(out=sel, in_=sel_d[:])
        nc.sync.dma_start(out=iota, in_=iota_d[:])
        nc.sync.dma_start(out=iotab, in_=iotab_d[:])
        nc.sync.dma_start(out=lnw, in_=lnw_d[:])
        nc.sync.dma_start(out=ident, in_=ident_d[:])
        # warm the Sqrt activation table before the timed pipeline needs it
        warm_t = consts.tile([128, 1], FP, tag="warm")
        nc.scalar.activation(warm_t, eps_t, mybir.ActivationFunctionType.Sqrt,
                             bias=eps_t, scale=1.0)

        lnw_b = lnw.unsqueeze(1).broadcast_to([128, 20, 8])
        iota_b = iota.unsqueeze(1).broadcast_to([128, 20, 8])
        iotab_b = iotab.unsqueeze(1).broadcast_to([128, 20, 8])

        # ---- input loads (quarter-split for DMA/compute pipelining) --------
        # Image 0 streams on the sync HWDGE ring, image 1 on the scalar
        # ring; each ring is FIFO per issuing engine, so the per-image
        # shift DMAs below never queue behind the other image's input.
        xeng = [nc.sync, nc.scalar]
        xt = []
        for b in range(BPC):
            row = []
            for kt in range(4):
                t = xpool.tile([128, HW], FR, tag="x")
                row.append(t)
            xt.append(row)

        # packed layout: 5 row-groups of 25 rows (17 sel outputs + 8 gumbel);
        # row-group rg covers pixels [512*rg, 512*rg+512)
        packs = []
        for b in range(BPC):
            packed = work.tile([125, 512], FP, tag="packed")
            packs.append(packed)
            # pixel blocks 18,19 don't exist; zero them so the vector
            # pipeline sees benign values in the padded lanes (rows 96:100
            # are re-covered by the rg=3 gumbel DMA afterwards)
            nc.scalar.memzero(packed[96:125, 256:512])
            for rg, (off, ncol) in enumerate(PCHUNKS):
                nc.gpsimd.dma_start(
                    out=packed[25 * rg + 17:25 * rg + 25, 0:ncol],
                    in_=g_d[b, :, off:off + ncol])

        for b in range(BPC):
            for q in range(4):
                for kt in range(4):
                    xeng[b].dma_start(
                        out=xt[b][kt][:, q * 576:(q + 1) * 576],
                        in_=x_d[b, kt * 128:(kt + 1) * 128,
                                q * 576:(q + 1) * 576])

        for b in range(BPC):
            # ---- conv as 1x1 matmul in fp32r, row-aligned chunks ----------
            # gridF: flat zero-padded layout, row pitch 49 (spacer column
            # absorbs horizontal shift wrap; head/tail absorb vertical)
            gridF = work.tile([81, GTOT], FR, tag="grid")
            nc.scalar.memzero(gridF[:, 0:GBASE])
            nc.scalar.memzero(gridF[:, GBASE + GP * 48:GTOT])
            nc.scalar.memzero(
                gridF[:, GBASE:GBASE + GP * 48].rearrange(
                    "p (r w) -> p r w", w=GP)[:, :, 48:49])
            for (r0, nr) in CCHUNKS:
                ncol = nr * 48
                P = psA.tile([81, 480], FP, tag="A")
                for kt in range(4):
                    nc.tensor.matmul(
                        P[:, 0:ncol],
                        w73[:, kt, :],
                        xt[b][kt][:, r0 * 48:r0 * 48 + ncol],
                        start=(kt == 0),
                        stop=(kt == 3),
                    )
                nc.scalar.copy(
                    out=gridF[:, GBASE + GP * r0:GBASE + GP * (r0 + nr)]
                    .rearrange("p (r w) -> p r w", w=GP)[:, :, 0:48],
                    in_=P[:, 0:ncol].rearrange("p (h w) -> p h w", w=W),
                )

            # ---- shifted per-tap rows: one contiguous run per partition --
            psh = work.tile([81, PSHW], FR, tag="psh")
            for t in range(9):
                ti, tj = divmod(t, 3)
                s = GBASE + GP * (ti - 1) + (tj - 1)
                xeng[b].dma_start(
                    out=psh[t * 9:(t + 1) * 9, :],
                    in_=gridF[t * 9:(t + 1) * 9, s:s + PSHW],
                )
            pshv = psh.rearrange("p (r w) -> p r w", w=GP)

            # ---- selection matmul (fp32r): rows 0-7 logits, 8+t = Sp_t ----
            # compute engines can only start at partition 0/32/64/96, so
            # results go PSUM -> whole-image stage -> (DMA) packed row 25*rg
            packed = packs[b]
            stage = work.tile([17, HW], FP, tag="stage")
            for (r0, nr) in CCHUNKS:
                ncol = nr * 48
                Lc = psS.tile([17, 480], FP, tag="S")
                nc.tensor.matmul(Lc[:, 0:ncol], sel,
                                 pshv[:, r0:r0 + nr, 0:48],
                                 start=True, stop=True)
                nc.scalar.copy(out=stage[:, r0 * 48:r0 * 48 + ncol],
                               in_=Lc[:, 0:ncol])
            for rg, (off, ncol) in enumerate(PCHUNKS):
                nc.gpsimd.dma_start(
                    out=packed[25 * rg:25 * rg + 17, 0:ncol],
                    in_=stage[:, off:off + ncol])

            # ---- transpose to pixel-partition layout (exact fp32) ---------
            # tt free layout [rg, cg, 25] so block index k = 4*rg+cg equals
            # the pixel-block index -> output rows come out in order
            tt = work.tile([128, 5, 4, 25], FP, tag="tt")
            for cg in range(4):
                tp = psT.tile([128, 125], FP, tag="T")
                nc.tensor.transpose(tp,
                                    packed[:, cg * 128:(cg + 1) * 128],
                                    ident[0:125, 0:125])
                nc.vector.tensor_copy(
                    tt[:, :, cg, :],
                    tp.rearrange("p (rg j) -> p rg j", j=25))
            ttv = tt.rearrange("p rg cg j -> p (rg cg) j")

            # ---- per-pixel LN + gumbel + argmax + neighbor select ---------
            Lap = ttv[:, :, 0:8]
            mu = vp.tile([128, 20], FP, tag="mu")
            nc.vector.tensor_reduce(mu, Lap, axis=AX.X, op=AL.add)
            cen = vp.tile([128, 20, 8], FP, tag="cen")
            nc.vector.scalar_tensor_tensor(
                cen, in0=mu.unsqueeze(2).broadcast_to([128, 20, 8]),
                scalar=-1.0 / 8.0, in1=Lap, op0=AL.mult, op1=AL.add)
            sq = vp.tile([128, 20, 8], FP, tag="sq")
            nc.vector.tensor_tensor(sq, cen, cen, op=AL.mult)
            v8 = vp.tile([128, 20], FP, tag="v8")
            nc.vector.tensor_reduce(v8, sq, axis=AX.X, op=AL.add)
            sd = vp.tile([128, 20], FP, tag="sd")
            nc.scalar.activation(sd, v8, mybir.ActivationFunctionType.Sqrt,
                                 bias=eps_t, scale=1.0 / 8.0)
            rstd = vp.tile([128, 20], FP, tag="rstd")
            nc.vector.reciprocal(rstd, sd)
            rl = vp.tile([128, 20, 8], FP, tag="rl")
            nc.vector.tensor_tensor(
                rl, rstd.unsqueeze(2).broadcast_to([128, 20, 8]), lnw_b,
                op=AL.mult)
            z = vp.tile([128, 20, 8], FP, tag="z")
            nc.vector.tensor_tensor(z, cen, rl, op=AL.mult)
            # add gumbel (+ln_bias, folded on host)
            z2 = vp.tile([128, 20, 8], FP, tag="z2")
            nc.vector.tensor_tensor(z2, z, ttv[:, :, 17:25], op=AL.add)

            mx = vp.tile([128, 20], FP, tag="mx")
            nc.vector.tensor_reduce(mx, z2, axis=AX.X, op=AL.max)
            eq = vp.tile([128, 20, 8], FP, tag="eq")
            nc.vector.tensor_tensor(
                eq, z2, mx.unsqueeze(2).broadcast_to([128, 20, 8]),
                op=AL.is_equal)
            im = vp.tile([128, 20, 8], FP, tag="im")
            nc.vector.scalar_tensor_tensor(
                im, in0=eq, scalar=-BIG, in1=iotab_b, op0=AL.mult, op1=AL.add)
            am = vp.tile([128, 20], FP, tag="am")
            nc.vector.tensor_reduce(am, im, axis=AX.X, op=AL.min)
            hard = vp.tile([128, 20, 8], FP, tag="hard")
            nc.vector.tensor_tensor(
                hard, iota_b, am.unsqueeze(2).broadcast_to([128, 20, 8]),
                op=AL.is_equal)

            # pair one-hot lanes with the 8 non-center taps (skip center=12)
            prod = vp.tile([128, 20, 8], FP, tag="prod")
            nc.vector.tensor_tensor(prod[:, :, 0:4], hard[:, :, 0:4],
                                    ttv[:, :, 8:12], op=AL.mult)
            nc.vector.tensor_tensor(prod[:, :, 4:8], hard[:, :, 4:8],
                                    ttv[:, :, 13:17], op=AL.mult)
            selS = vp.tile([128, 20], FP, tag="selS")
            nc.vector.tensor_reduce(selS, prod, axis=AX.X, op=AL.add)
            outm = vp.tile([128, 20], FP, tag="outm")
            nc.vector.tensor_tensor(outm, ttv[:, :, 12], selS,
                                    op=AL.subtract)

            # ---- transpose map back to pixel order and store --------------
            po = psO.tile([20, 128], FP, tag="O")
            nc.tensor.transpose(po, outm, ident)
            outsb = vp.tile([20, 128], FP, tag="outsb")
            nc.scalar.copy(out=outsb, in_=po)
            nc.gpsimd.dma_start(
                out=out_d[b, :].rearrange("(j f) -> j f", f=128),
                in_=outsb[0:18, :])

    nc.compile()
    return nc


def _to_fp32r(a):
    """Round fp32 -> fp32r (11-bit mantissa, low 12 bits zero, RNE).

    The PE's fp32r datapath reads only the top 20 bits; pre-rounding on
    the host lets the kernel DMA the data straight into float32r tiles.
    """
    u = np.ascontiguousarray(a, dtype=np.float32).view(np.uint32)
    r = (u + 0x7FF + ((u >> 12) & 1)) & np.uint32(0xFFFFF000)
    return r.view(np.float32)


def host_inputs(x, mask_weight, ln_weight, ln_bias, gumbel_noise):
    """Build per-core input maps (numpy only)."""
    x = _to_fp32r(np.asarray(x, dtype=np.float32)).reshape(B, C, HW)
    g = np.asarray(gumbel_noise, dtype=np.float32).reshape(B, 8, HW)
    # fold the LN bias into the (precomputed) gumbel noise
    g = np.ascontiguousarray(
        g + np.asarray(ln_bias, np.float32).reshape(1, 8, 1))

    mw = np.asarray(mask_weight, dtype=np.float32).reshape(8, C, 9)
    a = mw.transpose(1, 2, 0)                         # [c, tap, o]
    w73 = np.ones((C, 9, 9), dtype=np.float32)        # [c, tap, o|ones]
    w73[:, :, :8] = a
    w73 = w73.reshape(4, 128, 81).transpose(1, 0, 2)  # [c_mod, kt, m]
    w73 = _to_fp32r(w73)

    sel = np.zeros((81, 17), dtype=np.float32)
    for t in range(9):
        for o in range(8):
            sel[t * 9 + o, o] = 1.0
        sel[t * 9 + 8, 8 + t] = 1.0
    iota = np.broadcast_to(np.arange(8, dtype=np.float32), (128, 8)).copy()
    iotab = iota + BIG
    lnw = np.broadcast_to(
        np.asarray(ln_weight, np.float32).reshape(8), (128, 8)).copy()
    ident = np.eye(128, dtype=np.float32)

    shared = dict(w73=w73, sel=sel, iota=iota, iotab=iotab, lnw=lnw,
                  ident=ident)
    in_maps = []
    for c in range(N_CORES):
        m = dict(shared)
        m["x"] = np.ascontiguousarray(x[c * BPC:(c + 1) * BPC])
        m["g"] = np.ascontiguousarray(g[c * BPC:(c + 1) * BPC])
        in_maps.append(m)
    return in_maps


_NC = None


def kernel(x, mask_weight, ln_weight, ln_bias, gumbel_noise, init_flag=None,
           **_ignored):
    global _NC
    from concourse.bass_utils import run_bass_kernel_spmd

    if _NC is None:
        _NC = build_nc()
    in_maps = host_inputs(x, mask_weight, ln_weight, ln_bias, gumbel_noise)
    res = run_bass_kernel_spmd(_NC, in_maps, list(range(N_CORES))).results

    # all 512 output channels are identical: broadcast the per-image map
    out = np.empty((B, C, H, W), dtype=np.float32)
    for c in range(N_CORES):
        maps = res[c]["out"].reshape(BPC, H, W)
        out[c * BPC:(c + 1) * BPC] = maps[:, None, :, :]
    return out
